# revision 1
# baseline (speedup 1.0000x reference)
"""Causal self-attention (B=2, T=2048, C=1024, H=16) on 8 TRN2 NeuronCores.

Sharding: 8 cores = 2 batches x 4 head-groups (4 heads each).
Each core computes qkv projection for its heads, attention, and a partial
output projection (its rows of w_proj); the host sums the 4 partials per
batch and adds b_proj.

Device-side layout choices:
  - x is fed transposed (xT [C, T]) so the contraction dim C sits on SBUF
    partitions for all projection matmuls.
  - q, k are produced transposed ([d, t], heads stacked on partitions) so
    scores are computed TRANSPOSED: S^T[j, i] = kT.T @ qT.  Softmax over j
    (partitions) then needs no reductions: the AV matmul with a
    ones-augmented V yields the denominators as an extra output row.
  - v is produced natural ([t, d]) directly via a second projection.
  - All matmuls use float32r (full PE rate, ~12-bit mantissa); softmax exp
    runs in fp32 on the scalar engine (scale=1/8 folded in; no
    max-subtraction: scores are O(+-10) so exp cannot overflow in fp32).
  - Causal mask: gpsimd affine_select zeroes j>i entries of exp(S^T) on
    diagonal tiles only.
"""

import sys
import os

for _p in ("/opt/trn_rl_repo", "/root/.axon_site/_ro/trn_rl_repo"):
    if os.path.isdir(_p) and _p not in sys.path:
        sys.path.insert(0, _p)

import numpy as np
import concourse.bass as bass
import concourse.mybir as mybir
import concourse.tile as tile
from concourse import bacc
from concourse.bass_utils import run_bass_kernel_spmd

B, T, C, H = 2, 2048, 1024, 16
HS = C // H          # 64
HALF = HS // 2       # 32
NCORES = 8
NH = 4               # heads per core
TCH = 512            # t-chunk for projections / i-chunk for attention
CB = C // 128        # 8 contraction blocks
NTB = T // 128       # 16 t/j blocks
F32 = mybir.dt.float32
F32R = mybir.dt.float32r
BF16 = mybir.dt.bfloat16
USE_BF16 = os.environ.get("KERNEL_BF16", "0") == "1"
MMD = BF16 if USE_BF16 else F32R
AF = mybir.ActivationFunctionType
ALU = mybir.AluOpType

_CACHED = {}


def _build_nc():
    nc = bacc.Bacc("TRN2", target_bir_lowering=False, debug=False)

    xt = nc.dram_tensor("xt", [C, T], MMD, kind="ExternalInput").ap()
    wqk = nc.dram_tensor("wqk", [C, 512], MMD, kind="ExternalInput").ap()
    wv = nc.dram_tensor("wv", [C, 256], MMD, kind="ExternalInput").ap()
    wproj = nc.dram_tensor("wproj", [256, C], MMD, kind="ExternalInput").ap()
    bqk = nc.dram_tensor("bqk", [4, 128], F32, kind="ExternalInput").ap()
    cosrep = nc.dram_tensor("cosrep", [128, T], F32, kind="ExternalInput").ap()
    sinsw = nc.dram_tensor("sinsw", [128, T], F32, kind="ExternalInput").ap()
    ones64 = nc.dram_tensor("ones64", [128, 64], F32, kind="ExternalInput").ap()
    yout = nc.dram_tensor("yout", [T, C], F32, kind="ExternalOutput").ap()

    with tile.TileContext(nc) as tc:
        with (
            tc.tile_pool(name="const", bufs=1) as const,
            tc.tile_pool(name="persist", bufs=1) as persist,
            tc.tile_pool(name="work", bufs=2) as work,
            tc.tile_pool(name="ps", bufs=1, space="PSUM") as ps,
        ):
            # ---- constant loads (wqk/xt first; one DMA per cb so the first
            # matmuls can start on partial loads) ---------------------------------
            xtp_cm = tc.tile_pool(name="xtp", bufs=2)
            xtp = xtp_cm.__enter__()
            wqk_sb = const.tile([128, CB * 512], MMD)
            xt0_t = xtp.tile([128, CB * TCH], MMD, tag="xt", bufs=3, name="xt0")
            for cb in range(CB):
                nc.scalar.dma_start(
                    out=wqk_sb[:, cb * 512 : (cb + 1) * 512],
                    in_=wqk[cb * 128 : (cb + 1) * 128, :],
                )
                nc.sync.dma_start(
                    out=xt0_t[:, cb * TCH : (cb + 1) * TCH],
                    in_=xt[cb * 128 : (cb + 1) * 128, 0:TCH],
                )
            wv_sb = const.tile([128, CB * 256], MMD)
            nc.scalar.dma_start(
                out=wv_sb.rearrange("p (cb m) -> p cb m", cb=CB),
                in_=wv.rearrange("(cb p) m -> p cb m", p=128),
            )
            # ---- persistent intermediates --------------------------------------
            qt_sb = persist.tile([128, 2 * T], MMD)   # [Q01 | Q23], [d(2 heads), t]
            kt_sb = persist.tile([128, 2 * T], MMD)
            v_sb = persist.tile([128, NTB * 260], MMD)  # per j-block: 4x(64 v + 1 one)
            ctx0 = persist.tile([128, T], MMD)        # heads 0,1 ctxT
            ctx1 = persist.tile([128, T], MMD)        # heads 2,3 ctxT

            cos_sb = const.tile([128, T], F32)
            nc.scalar.dma_start(out=cos_sb, in_=cosrep)
            sin_sb = const.tile([128, T], F32)
            nc.scalar.dma_start(out=sin_sb, in_=sinsw)
            ones_sb = const.tile([128, 64], F32)
            nc.scalar.dma_start(out=ones_sb, in_=ones64)
            bqk_sb = const.tile([128, 4], F32)
            for mt in range(4):
                nc.scalar.dma_start(out=bqk_sb[:, mt : mt + 1], in_=bqk[mt, :][:, None])

            # ones columns of v_sb (once)
            nc.vector.tensor_copy(
                v_sb.rearrange("p (tb h d) -> p tb h d", tb=NTB, h=4)[:, :, :, 64:65],
                ones_sb.rearrange("p (a b c) -> p a b c", a=NTB, b=4),
            )

            # ---- phase 1: qkv projection + rope --------------------------------
            # q/k psums cover 1024 t-columns (2 banks); rope runs 1024-wide.
            xt_tiles = {}
            for tci2 in range(2):
                for half in range(2):
                    tci = tci2 * 2 + half
                    if tci == 0:
                        xt_tiles[0] = xt0_t
                        continue
                    xt_t = xtp.tile([128, CB * TCH], MMD, tag="xt", bufs=3, name=f"xt{tci}")
                    for cb in range(CB):
                        nc.sync.dma_start(
                            out=xt_t[:, cb * TCH : (cb + 1) * TCH],
                            in_=xt[cb * 128 : (cb + 1) * 128, tci * TCH : (tci + 1) * TCH],
                        )
                    xt_tiles[tci] = xt_t
                tsl2 = slice(tci2 * 1024, (tci2 + 1) * 1024)

                # q/k M-tiles: 0=Q(h0,h1) 1=Q(h2,h3) 2=K(h0,h1) 3=K(h2,h3)
                for mt in range(4):
                    pq = ps.tile([128, 1024], F32, tag="pa", bufs=3, name=f"pq{tci2}_{mt}")
                    for half in range(2):
                        xt_t = xt_tiles[tci2 * 2 + half]
                        for cb in range(CB):
                            nc.tensor.matmul(
                                pq[:, half * TCH : (half + 1) * TCH],
                                lhsT=wqk_sb[:, cb * 512 + mt * 128 : cb * 512 + (mt + 1) * 128],
                                rhs=xt_t[:, cb * TCH : (cb + 1) * TCH],
                                start=(cb == 0),
                                stop=(cb == CB - 1),
                            )
                    # rope: out = (pq+b)*cos + swap(pq+b)*sin.  Two psum
                    # readers only (qb eviction + m1) so the pa slot frees fast.
                    m1 = work.tile([128, 1024], F32, tag="m1", bufs=1, name=f"m1_{tci2}_{mt}")
                    nc.vector.scalar_tensor_tensor(
                        out=m1, in0=pq, scalar=bqk_sb[:, mt : mt + 1],
                        in1=cos_sb[:, tsl2], op0=ALU.add, op1=ALU.mult,
                    )
                    qb = work.tile([128, 1024], F32, tag="qb", name=f"qb{tci2}_{mt}")
                    nc.scalar.activation(
                        qb, pq, AF.Identity, bias=bqk_sb[:, mt : mt + 1], scale=1.0
                    )
                    swp = work.tile([128, 1024], F32, tag="swp", bufs=1, name=f"swp{tci2}_{mt}")
                    for dst0, src0 in ((0, 32), (32, 0), (64, 96), (96, 64)):
                        nc.vector.tensor_mul(
                            swp[dst0 : dst0 + 32, :],
                            qb[src0 : src0 + 32, :],
                            sin_sb[src0 : src0 + 32, tsl2],
                        )
                    dest = qt_sb if mt < 2 else kt_sb
                    dcol = (mt % 2) * T + tci2 * 1024
                    nc.vector.tensor_add(dest[:, dcol : dcol + 1024], m1, swp)

                # v projection: natural layout [t, d], 4 t-blocks per psum tile
                pv = ps.tile([128, 1024], F32, tag="pa", bufs=3, name=f"pv{tci2}_a")
                pv2 = ps.tile([128, 1024], F32, tag="pa", bufs=3, name=f"pv{tci2}_b")
                for sub in range(8):
                    tb = tci2 * 8 + sub
                    dst = pv if sub < 4 else pv2
                    q = sub % 4
                    xt_t = xt_tiles[tb // 4]
                    for cb in range(CB):
                        nc.tensor.matmul(
                            dst[:, q * 256 : (q + 1) * 256],
                            lhsT=xt_t[:, cb * TCH + (tb % 4) * 128 : cb * TCH + (tb % 4 + 1) * 128],
                            rhs=wv_sb[:, cb * 256 : (cb + 1) * 256],
                            start=(cb == 0),
                            stop=(cb == CB - 1),
                        )
                    nc.scalar.copy(
                        v_sb[:, tb * 260 : tb * 260 + 260].rearrange(
                            "p (h d) -> p h d", h=4
                        )[:, :, 0:64],
                        dst[:, q * 256 : (q + 1) * 256].rearrange(
                            "p (h d) -> p h d", h=4
                        ),
                    )

            xtp_cm.__exit__(None, None, None)
            attnp_cm = tc.tile_pool(name="attnp", bufs=6)
            attnp = attnp_cm.__enter__()

            # ---- phase 2: attention (i-chunk outer, head-pair inner) -----------
            for ici in range(T // TCH):
                isl = slice(ici * TCH, (ici + 1) * TCH)
                njb = 4 * (ici + 1)
                for pair in range(2):
                    qt_p = qt_sb[:, pair * T : (pair + 1) * T]
                    kt_p = kt_sb[:, pair * T : (pair + 1) * T]
                    ctx_p = ctx0 if pair == 0 else ctx1
                    ctxps = [
                        ps.tile([65, 512], F32, tag="ctx", bufs=2, name=f"ctxp{pair}_{ici}_{hh}")
                        for hh in range(2)
                    ]

                    def emit_av(at_pair, duo):
                        for hh in range(2):
                            for half in range(2):
                                jb = duo * 2 + half
                                h_loc = pair * 2 + hh
                                nc.tensor.matmul(
                                    ctxps[hh],
                                    lhsT=v_sb[:, jb * 260 + h_loc * 65 : jb * 260 + (h_loc + 1) * 65],
                                    rhs=at_pair[hh][:, half * 512 : (half + 1) * 512],
                                    start=(jb == 0),
                                    stop=(jb == njb - 1),
                                )

                    pending = []
                    for duo in range(njb // 2):
                        st = [
                            ps.tile([128, 1024], F32, tag="pa", bufs=3, name=f"st{pair}_{ici}_{duo}_{hh}")
                            for hh in range(2)
                        ]
                        # interleave the two heads' QK matmuls (disjoint row
                        # strips 0-63 / 64-127 -> HW runs them concurrently)
                        for half in range(2):
                            jb = duo * 2 + half
                            for hh in range(2):
                                nc.tensor.matmul(
                                    st[hh][:, half * 512 : (half + 1) * 512],
                                    lhsT=kt_p[hh * 64 : (hh + 1) * 64, jb * 128 : (jb + 1) * 128],
                                    rhs=qt_p[hh * 64 : (hh + 1) * 64, isl],
                                    start=True,
                                    stop=True,
                                )
                        if len(pending) >= 1:
                            emit_av(*pending.pop(0))
                        at_pair = []
                        for hh in range(2):
                            at = attnp.tile([128, 1024], MMD, tag="attn", bufs=6, name=f"at{pair}_{ici}_{duo}_{hh}")
                            nc.scalar.activation(at, st[hh], AF.Exp, scale=0.125)
                            if duo * 2 >= 4 * ici:  # diagonal duo: zero j > i, both halves
                                nc.gpsimd.affine_select(
                                    out=at,
                                    in_=at,
                                    compare_op=ALU.is_ge,
                                    fill=0.0,
                                    base=ici * TCH - duo * 2 * 128,
                                    channel_multiplier=-1,
                                    pattern=[[-128, 2], [1, 512]],
                                )
                            at_pair.append(at)
                        pending.append((at_pair, duo))
                    for p in pending:
                        emit_av(*p)

                    # normalize: ctx[d, i] /= denom[i] (denom = row 64)
                    for hh in range(2):
                        # single-op psum eviction: ctx slot frees immediately;
                        # normalization continues SBUF-side off the PE path
                        ctxu = work.tile([64, 512], F32, tag="ctxu", bufs=2, name=f"cu{pair}_{ici}_{hh}")
                        nc.vector.tensor_copy(ctxu, ctxps[hh][0:64, :])
                        dn = work.tile([1, 512], F32, tag="dnrow", bufs=2, name=f"dn{pair}_{ici}_{hh}")
                        nc.vector.tensor_copy(dn, ctxps[hh][64:65, :])
                        rc = work.tile([1, 512], F32, tag="recip", bufs=1, name=f"rc{pair}_{ici}_{hh}")
                        nc.vector.reciprocal_approx_fast(out=rc, in_=dn)
                        bcast = work.tile([64, 512], F32, tag="bcast", bufs=2, name=f"bcast{pair}_{ici}_{hh}")
                        nc.gpsimd.partition_broadcast(bcast, rc)
                        nc.vector.tensor_mul(
                            ctx_p[hh * 64 : (hh + 1) * 64, isl],
                            ctxu,
                            bcast,
                        )

            attnp_cm.__exit__(None, None, None)

            # ---- phase 3: output projection (partial: our 256 rows of w_proj) --
            wproj_sb = const.tile([128, 2 * C], MMD)
            nc.sync.dma_start(
                out=wproj_sb.rearrange("p (cb n) -> p cb n", cb=2),
                in_=wproj.rearrange("(cb p) n -> p cb n", p=128),
            )
            for tb in range(NTB):
                yp = ps.tile([128, 1024], F32, tag="pa", bufs=3, name=f"yp{tb}")
                for ncol in range(2):
                    for cb in range(2):
                        ctx_t = ctx0 if cb == 0 else ctx1
                        nc.tensor.matmul(
                            yp[:, ncol * 512 : (ncol + 1) * 512],
                            lhsT=ctx_t[:, tb * 128 : (tb + 1) * 128],
                            rhs=wproj_sb[:, cb * C + ncol * 512 : cb * C + (ncol + 1) * 512],
                            start=(cb == 0),
                            stop=(cb == 1),
                        )
                ysb = work.tile([128, 1024], F32, tag="ysb", bufs=2, name=f"ysb{tb}")
                nc.vector.tensor_copy(ysb, yp)
                nc.sync.dma_start(out=yout[tb * 128 : (tb + 1) * 128, :], in_=ysb)

    nc.compile()
    return nc


def _prep_core_inputs(x, cos, sin, w_attn, b_attn, w_proj):
    """Build the 8 per-core input maps (host-side shard/reorder)."""
    import ml_dtypes
    mmnp = ml_dtypes.bfloat16 if USE_BF16 else np.float32
    x = np.asarray(x, dtype=np.float32)
    cos = np.asarray(cos, dtype=np.float32).reshape(T, HALF)
    sin = np.asarray(sin, dtype=np.float32).reshape(T, HALF)
    w_attn = np.asarray(w_attn, dtype=np.float32)
    b_attn = np.asarray(b_attn, dtype=np.float32)
    w_proj = np.asarray(w_proj, dtype=np.float32)

    cosT = np.ascontiguousarray(cos.T)               # [32, T]
    sinT = np.ascontiguousarray(sin.T)
    cosrep = np.tile(cosT, (4, 1))                   # [128, T]
    sin_sw = np.concatenate([sinT, -sinT, sinT, -sinT], axis=0)
    ones64 = np.ones((128, 64), np.float32)

    xts = [np.ascontiguousarray(x[b].T).astype(mmnp) for b in range(B)]  # [C, T] each

    in_maps = []
    for core in range(NCORES):
        b = core // 4
        g = core % 4
        heads = [4 * g + i for i in range(NH)]
        # q/k column blocks: M-tiles [Q(h0,h1), Q(h2,h3), K(h0,h1), K(h2,h3)]
        qcols, bq = [], []
        for mt, (base, hs) in enumerate(
            [(0, heads[0:2]), (0, heads[2:4]), (C, heads[0:2]), (C, heads[2:4])]
        ):
            cols = np.concatenate([np.arange(base + h * HS, base + (h + 1) * HS) for h in hs])
            qcols.append(cols)
            bq.append(b_attn[cols])
        wqk_c = np.ascontiguousarray(w_attn[:, np.concatenate(qcols)]).astype(mmnp)
        bqk_c = np.stack(bq)                                            # [4, 128]
        vcols = np.concatenate(
            [np.arange(2 * C + h * HS, 2 * C + (h + 1) * HS) for h in heads]
        )
        wv_c = np.ascontiguousarray(w_attn[:, vcols]).astype(mmnp)
        wproj_c = np.ascontiguousarray(w_proj[g * 256 : (g + 1) * 256, :]).astype(mmnp)
        in_maps.append(
            {
                "xt": xts[b],
                "wqk": wqk_c,
                "wv": wv_c,
                "wproj": wproj_c,
                "bqk": np.ascontiguousarray(bqk_c),
                "cosrep": np.ascontiguousarray(cosrep),
                "sinsw": np.ascontiguousarray(sin_sw),
                "ones64": ones64,
            }
        )
    return in_maps


def kernel(x, cos, sin, w_attn, b_attn, w_proj, b_proj, _want_trace=False):
    if "nc" not in _CACHED:
        _CACHED["nc"] = _build_nc()
    nc = _CACHED["nc"]
    in_maps = _prep_core_inputs(x, cos, sin, w_attn, b_attn, w_proj)
    res = run_bass_kernel_spmd(
        nc, in_maps, core_ids=list(range(NCORES)), trace=_want_trace
    )
    _CACHED["last_result"] = res
    b_proj = np.asarray(b_proj, dtype=np.float32)
    # v-bias folds out of attention (softmax rows sum to 1): it contributes a
    # constant b_v @ w_proj to every output row, added here with b_proj.
    bv = np.asarray(b_attn, dtype=np.float32)[2 * C : 3 * C]
    bias_full = b_proj + bv @ np.asarray(w_proj, dtype=np.float32)
    out = np.empty((B, T, C), np.float32)
    for b in range(B):
        acc = res.results[b * 4]["yout"].astype(np.float32).copy()
        for g in range(1, 4):
            acc += res.results[b * 4 + g]["yout"]
        out[b] = acc + bias_full[None, :]
    return out



# revision 5
# speedup vs baseline: 1.2098x; 1.2098x over previous
"""Causal self-attention (B=2, T=2048, C=1024, H=16) on 8 TRN2 NeuronCores.

Sharding: 8 cores = 2 batches x 4 head-groups (4 heads each).
Each core computes qkv projection for its heads, attention, and a partial
output projection (its rows of w_proj); the host sums the 4 partials per
batch and adds b_proj.

Device-side structure (v2, fused pipeline):
  - One software-pipelined loop over four 512-token chunks.  For chunk ci
    the attention i-chunk (scores -> exp -> mask -> AV) is emitted on the
    tensor queue INTERLEAVED with the qkv projection of chunk ci+1 (and,
    for the last chunk, with the output projection of earlier chunks) so
    the PE never idles waiting for the scalar-engine exp and stays at its
    2.4 GHz p-state.
  - Layouts as v1: x fed transposed; q,k produced transposed ([d, t]) so
    scores are computed transposed and softmax needs no reductions (the
    AV matmul with a ones-augmented V yields the denominators); v natural.
  - Diagonal-tile trimming: matmuls/exp on diagonal j-blocks only cover
    the causal column range (c0 = 128k), so fully-masked regions are never
    computed; affine_select only handles the [128,128] partial blocks.
"""

import sys
import os

for _p in ("/opt/trn_rl_repo", "/root/.axon_site/_ro/trn_rl_repo"):
    if os.path.isdir(_p) and _p not in sys.path:
        sys.path.insert(0, _p)

import numpy as np
import concourse.bass as bass
import concourse.mybir as mybir
import concourse.tile as tile
from concourse import bacc
from concourse.bass_utils import run_bass_kernel_spmd

B, T, C, H = 2, 2048, 1024, 16
HS = C // H          # 64
HALF = HS // 2       # 32
NCORES = 8
NH = 4               # heads per core
TCH = 512            # t-chunk for projections / i-chunk for attention
NCH = T // TCH       # 4 chunks
CB = C // 128        # 8 contraction blocks
NTB = T // 128       # 16 t/j blocks
F32 = mybir.dt.float32
F32R = mybir.dt.float32r
BF16 = mybir.dt.bfloat16
USE_BF16 = os.environ.get("KERNEL_BF16", "0") == "1"
MMD = BF16 if USE_BF16 else F32R
AF = mybir.ActivationFunctionType
ALU = mybir.AluOpType

_CACHED = {}


def _build_nc():
    nc = bacc.Bacc("TRN2", target_bir_lowering=False, debug=False)

    xt = nc.dram_tensor("xt", [C, T], MMD, kind="ExternalInput").ap()
    wqk = nc.dram_tensor("wqk", [C, 512], MMD, kind="ExternalInput").ap()
    wv = nc.dram_tensor("wv", [C, 256], MMD, kind="ExternalInput").ap()
    wproj = nc.dram_tensor("wproj", [256, C], MMD, kind="ExternalInput").ap()
    bqk = nc.dram_tensor("bqk", [4, 128], F32, kind="ExternalInput").ap()
    cosrep = nc.dram_tensor("cosrep", [128, T], F32, kind="ExternalInput").ap()
    sinsw = nc.dram_tensor("sinsw", [128, T], F32, kind="ExternalInput").ap()
    ones64 = nc.dram_tensor("ones64", [128, 64], F32, kind="ExternalInput").ap()
    yout = nc.dram_tensor("yout", [T, C], F32, kind="ExternalOutput").ap()

    with tile.TileContext(nc) as tc:
        with (
            tc.tile_pool(name="const", bufs=1) as const,
            tc.tile_pool(name="persist", bufs=1) as persist,
            tc.tile_pool(name="work", bufs=2) as work,
            tc.tile_pool(name="xtp", bufs=2) as xtp,
            tc.tile_pool(name="attnp", bufs=6) as attnp,
            tc.tile_pool(name="ps", bufs=1, space="PSUM") as ps,
        ):
            # ---- constant loads (wqk/xt0 first, per-cb, so the first matmuls
            # can start on partial loads) ----------------------------------------
            wqk_sb = const.tile([128, CB * 512], MMD)
            xt_tiles = {}
            xt0_t = xtp.tile([128, CB * TCH], MMD, tag="xt", bufs=2, name="xt0")
            for cb in range(CB):
                nc.scalar.dma_start(
                    out=wqk_sb[:, cb * 512 : (cb + 1) * 512],
                    in_=wqk[cb * 128 : (cb + 1) * 128, :],
                )
                nc.sync.dma_start(
                    out=xt0_t[:, cb * TCH : (cb + 1) * TCH],
                    in_=xt[cb * 128 : (cb + 1) * 128, 0:TCH],
                )
            xt_tiles[0] = xt0_t
            wv_sb = const.tile([128, CB * 256], MMD)
            nc.scalar.dma_start(
                out=wv_sb.rearrange("p (cb m) -> p cb m", cb=CB),
                in_=wv.rearrange("(cb p) m -> p cb m", p=128),
            )
            cos_sb = const.tile([128, T], F32)
            nc.scalar.dma_start(out=cos_sb, in_=cosrep)
            sin_sb = const.tile([128, T], F32)
            nc.scalar.dma_start(out=sin_sb, in_=sinsw)
            ones_sb = const.tile([128, 64], F32)
            nc.scalar.dma_start(out=ones_sb, in_=ones64)
            bqk_sb = const.tile([128, 4], F32)
            for mt in range(4):
                nc.scalar.dma_start(out=bqk_sb[:, mt : mt + 1], in_=bqk[mt, :][:, None])
            wproj_sb = const.tile([128, 2 * C], MMD)
            nc.scalar.dma_start(
                out=wproj_sb.rearrange("p (cb n) -> p cb n", cb=2),
                in_=wproj.rearrange("(cb p) n -> p cb n", p=128),
            )

            # ---- persistent intermediates --------------------------------------
            qt_sb = persist.tile([128, 2 * T], MMD)   # [Q01 | Q23], [d(2 heads), t]
            kt_sb = persist.tile([128, 2 * T], MMD)
            v_sb = persist.tile([128, NTB * 260], MMD)  # per j-block: 4x(64 v + 1 one)
            ctx0 = persist.tile([128, T], MMD)        # heads 0,1 ctxT
            ctx1 = persist.tile([128, T], MMD)        # heads 2,3 ctxT

            # ones columns of v_sb (once)
            nc.vector.tensor_copy(
                v_sb.rearrange("p (tb h d) -> p tb h d", tb=NTB, h=4)[:, :, :, 64:65],
                ones_sb.rearrange("p (a b c) -> p a b c", a=NTB, b=4),
            )

            # ---- emission helpers ----------------------------------------------
            def emit_xt_dma(ci):
                xt_t = xtp.tile([128, CB * TCH], MMD, tag="xt", bufs=2, name=f"xt{ci}")
                for cb in range(CB):
                    nc.sync.dma_start(
                        out=xt_t[:, cb * TCH : (cb + 1) * TCH],
                        in_=xt[cb * 128 : (cb + 1) * 128, ci * TCH : (ci + 1) * TCH],
                    )
                xt_tiles[ci] = xt_t

            def emit_projqk_mt(ci, mt):
                # M-tiles: 0=Q(h0,h1) 1=Q(h2,h3) 2=K(h0,h1) 3=K(h2,h3)
                xt_t = xt_tiles[ci]
                tsl = slice(ci * TCH, (ci + 1) * TCH)
                pq = ps.tile([128, TCH], F32, tag="sm", bufs=2, name=f"pq{ci}_{mt}")
                for cb in range(CB):
                    nc.tensor.matmul(
                        pq,
                        lhsT=wqk_sb[:, cb * 512 + mt * 128 : cb * 512 + (mt + 1) * 128],
                        rhs=xt_t[:, cb * TCH : (cb + 1) * TCH],
                        start=(cb == 0),
                        stop=(cb == CB - 1),
                    )
                # rope: out = (pq+b)*cos + swap(pq+b)*sin.  Two psum readers
                # only (qb eviction + m1) so the sm slot frees fast.
                m1 = work.tile([128, TCH], F32, tag="m1", bufs=2, name=f"m1_{ci}_{mt}")
                nc.vector.scalar_tensor_tensor(
                    out=m1, in0=pq, scalar=bqk_sb[:, mt : mt + 1],
                    in1=cos_sb[:, tsl], op0=ALU.add, op1=ALU.mult,
                )
                qb = work.tile([128, TCH], F32, tag="qb", bufs=2, name=f"qb{ci}_{mt}")
                nc.scalar.activation(
                    qb, pq, AF.Identity, bias=bqk_sb[:, mt : mt + 1], scale=1.0
                )
                swp = work.tile([128, TCH], F32, tag="swp", bufs=2, name=f"swp{ci}_{mt}")
                for dst0, src0 in ((0, 32), (32, 0), (64, 96), (96, 64)):
                    nc.vector.tensor_mul(
                        swp[dst0 : dst0 + 32, :],
                        qb[src0 : src0 + 32, :],
                        sin_sb[src0 : src0 + 32, tsl],
                    )
                dest = qt_sb if mt < 2 else kt_sb
                dcol = (mt % 2) * T + ci * TCH
                nc.vector.tensor_add(dest[:, dcol : dcol + TCH], m1, swp)

            def emit_projv_half(ci, h):
                # v natural layout [t, d]: two 128-t-blocks per psum tile
                xt_t = xt_tiles[ci]
                pv = ps.tile([128, TCH], F32, tag="sm", bufs=2, name=f"pv{ci}_{h}")
                for sub in range(2):
                    tl = h * 2 + sub            # t-block within chunk
                    tb = ci * 4 + tl            # global 128-t block
                    for cb in range(CB):
                        nc.tensor.matmul(
                            pv[:, sub * 256 : (sub + 1) * 256],
                            lhsT=xt_t[:, cb * TCH + tl * 128 : cb * TCH + (tl + 1) * 128],
                            rhs=wv_sb[:, cb * 256 : (cb + 1) * 256],
                            start=(cb == 0),
                            stop=(cb == CB - 1),
                        )
                for sub in range(2):
                    tb = ci * 4 + h * 2 + sub
                    nc.vector.tensor_copy(
                        v_sb[:, tb * 260 : tb * 260 + 260].rearrange(
                            "p (h d) -> p h d", h=4
                        )[:, :, 0:64],
                        pv[:, sub * 256 : (sub + 1) * 256].rearrange(
                            "p (h d) -> p h d", h=4
                        ),
                    )

            def emit_outproj_tb(tb):
                ysb = work.tile([128, 1024], F32, tag="ysb", bufs=2, name=f"ysb{tb}")
                for ncol in range(2):
                    yp = ps.tile([128, TCH], F32, tag="sm", bufs=2, name=f"yp{tb}_{ncol}")
                    for cb in range(2):
                        ctx_t = ctx0 if cb == 0 else ctx1
                        nc.tensor.matmul(
                            yp,
                            lhsT=ctx_t[:, tb * 128 : (tb + 1) * 128],
                            rhs=wproj_sb[:, cb * C + ncol * 512 : cb * C + (ncol + 1) * 512],
                            start=(cb == 0),
                            stop=(cb == 1),
                        )
                    nc.vector.tensor_copy(ysb[:, ncol * 512 : (ncol + 1) * 512], yp)
                nc.sync.dma_start(out=yout[tb * 128 : (tb + 1) * 128, :], in_=ysb)

            # ---- attention chunk with interleaved filler pieces ----------------
            def emit_attention_chunk(ci, fillers):
                npull = [0]
                nslots = 4 * (ci + 1)

                def pull(slot_done):
                    tgt = (slot_done * len(fillers) + nslots - 1) // nslots
                    while npull[0] < min(tgt, len(fillers)):
                        fillers[npull[0]]()
                        npull[0] += 1

                njb = 4 * (ci + 1)
                isl = slice(ci * TCH, (ci + 1) * TCH)
                for pair in range(2):
                    qt_p = qt_sb[:, pair * T : (pair + 1) * T]
                    kt_p = kt_sb[:, pair * T : (pair + 1) * T]
                    ctx_p = ctx0 if pair == 0 else ctx1
                    ctxps = [
                        ps.tile([65, TCH], F32, tag="ctx", bufs=2, name=f"cp{pair}_{ci}_{hh}")
                        for hh in range(2)
                    ]

                    def trim(jb):
                        # causal column trim for diagonal j-blocks, clamped to
                        # keep matmul free dim >= 256 (fp32r full-rate floor)
                        return min(max(0, 128 * (jb - 4 * ci)), 256)

                    def emit_av(at_pair, duo):
                        for hh in range(2):
                            for half in range(2):
                                jb = duo * 2 + half
                                c0 = trim(jb)
                                h_loc = pair * 2 + hh
                                nc.tensor.matmul(
                                    ctxps[hh][:, c0:TCH],
                                    lhsT=v_sb[:, jb * 260 + h_loc * 65 : jb * 260 + (h_loc + 1) * 65],
                                    rhs=at_pair[hh][:, half * 512 + c0 : (half + 1) * 512],
                                    start=(jb == 0),
                                    stop=(jb == njb - 1),
                                )

                    pending = []
                    for duo in range(njb // 2):
                        st = [
                            ps.tile([128, 1024], F32, tag="st", bufs=2, name=f"st{pair}_{ci}_{duo}_{hh}")
                            for hh in range(2)
                        ]
                        # interleave the two heads' QK matmuls (disjoint row
                        # strips 0-63 / 64-127 -> HW runs them concurrently)
                        for half in range(2):
                            jb = duo * 2 + half
                            c0 = trim(jb)
                            for hh in range(2):
                                nc.tensor.matmul(
                                    st[hh][:, half * 512 + c0 : (half + 1) * 512],
                                    lhsT=kt_p[hh * 64 : (hh + 1) * 64, jb * 128 : (jb + 1) * 128],
                                    rhs=qt_p[hh * 64 : (hh + 1) * 64, ci * TCH + c0 : (ci + 1) * TCH],
                                    start=True,
                                    stop=True,
                                )
                        if len(pending) >= 1:
                            emit_av(*pending.pop(0))
                        at_pair = []
                        for hh in range(2):
                            at = attnp.tile([128, 1024], MMD, tag="attn", bufs=6, name=f"at{pair}_{ci}_{duo}_{hh}")
                            if duo < 2 * ci:  # fully off-diagonal duo
                                nc.scalar.activation(at, st[hh], AF.Exp, scale=0.125)
                            else:
                                # diagonal: exp only the causal column ranges,
                                # then zero j>i up through the partial block
                                for half in range(2):
                                    jb = duo * 2 + half
                                    k = jb - 4 * ci
                                    c0 = trim(jb)
                                    nc.scalar.activation(
                                        at[:, half * 512 + c0 : (half + 1) * 512],
                                        st[hh][:, half * 512 + c0 : (half + 1) * 512],
                                        AF.Exp,
                                        scale=0.125,
                                    )
                                    sel = 128 * (k + 1) - c0
                                    nc.gpsimd.affine_select(
                                        out=at[:, half * 512 + c0 : half * 512 + c0 + sel],
                                        in_=at[:, half * 512 + c0 : half * 512 + c0 + sel],
                                        compare_op=ALU.is_ge,
                                        fill=0.0,
                                        base=c0 - 128 * k,
                                        channel_multiplier=-1,
                                        pattern=[[1, sel]],
                                    )
                            at_pair.append(at)
                        pending.append((at_pair, duo))
                        pull(pair * (njb // 2) + duo + 1)
                    for p in pending:
                        emit_av(*p)

                    # normalize: ctx[d, i] /= denom[i] (denom = row 64)
                    for hh in range(2):
                        # single-op psum eviction: ctx slot frees immediately;
                        # normalization continues SBUF-side off the PE path
                        ctxu = work.tile([64, TCH], F32, tag="ctxu", bufs=2, name=f"cu{pair}_{ci}_{hh}")
                        nc.vector.tensor_copy(ctxu, ctxps[hh][0:64, :])
                        dn = work.tile([1, TCH], F32, tag="dnrow", bufs=2, name=f"dn{pair}_{ci}_{hh}")
                        nc.vector.tensor_copy(dn, ctxps[hh][64:65, :])
                        rc = work.tile([1, TCH], F32, tag="recip", bufs=1, name=f"rc{pair}_{ci}_{hh}")
                        nc.vector.reciprocal_approx_fast(out=rc, in_=dn)
                        bcast = work.tile([64, TCH], F32, tag="bcast", bufs=2, name=f"bcast{pair}_{ci}_{hh}")
                        nc.gpsimd.partition_broadcast(bcast, rc)
                        nc.vector.tensor_mul(
                            ctx_p[hh * 64 : (hh + 1) * 64, isl],
                            ctxu,
                            bcast,
                        )
                pull(nslots)  # drain any leftover fillers

            # ---- fused pipeline ------------------------------------------------
            def proj_pieces(ci):
                return (
                    [lambda mt=mt: emit_projqk_mt(ci, mt) for mt in range(4)]
                    + [lambda h=h: emit_projv_half(ci, h) for h in range(2)]
                )

            # lead-in: chunk 0 projection runs alone
            for f in proj_pieces(0):
                f()
            for ci in range(NCH):
                if ci + 1 < NCH:
                    emit_xt_dma(ci + 1)
                    fillers = proj_pieces(ci + 1)
                else:
                    # last chunk: fill PE gaps with earlier chunks' out-proj
                    fillers = [
                        (lambda tb=tb: emit_outproj_tb(tb))
                        for tb in range((NCH - 1) * 4)
                    ]
                emit_attention_chunk(ci, fillers)
            for tb in range((NCH - 1) * 4, NCH * 4):
                emit_outproj_tb(tb)

    nc.compile()
    return nc


def _prep_core_inputs(x, cos, sin, w_attn, b_attn, w_proj):
    """Build the 8 per-core input maps (host-side shard/reorder)."""
    import ml_dtypes
    mmnp = ml_dtypes.bfloat16 if USE_BF16 else np.float32
    x = np.asarray(x, dtype=np.float32)
    cos = np.asarray(cos, dtype=np.float32).reshape(T, HALF)
    sin = np.asarray(sin, dtype=np.float32).reshape(T, HALF)
    w_attn = np.asarray(w_attn, dtype=np.float32)
    b_attn = np.asarray(b_attn, dtype=np.float32)
    w_proj = np.asarray(w_proj, dtype=np.float32)

    cosT = np.ascontiguousarray(cos.T)               # [32, T]
    sinT = np.ascontiguousarray(sin.T)
    cosrep = np.tile(cosT, (4, 1))                   # [128, T]
    sin_sw = np.concatenate([sinT, -sinT, sinT, -sinT], axis=0)
    ones64 = np.ones((128, 64), np.float32)

    xts = [np.ascontiguousarray(x[b].T).astype(mmnp) for b in range(B)]  # [C, T] each

    in_maps = []
    for core in range(NCORES):
        b = core // 4
        g = core % 4
        heads = [4 * g + i for i in range(NH)]
        # q/k column blocks: M-tiles [Q(h0,h1), Q(h2,h3), K(h0,h1), K(h2,h3)]
        qcols, bq = [], []
        for mt, (base, hs) in enumerate(
            [(0, heads[0:2]), (0, heads[2:4]), (C, heads[0:2]), (C, heads[2:4])]
        ):
            cols = np.concatenate([np.arange(base + h * HS, base + (h + 1) * HS) for h in hs])
            qcols.append(cols)
            bq.append(b_attn[cols])
        wqk_c = np.ascontiguousarray(w_attn[:, np.concatenate(qcols)]).astype(mmnp)
        bqk_c = np.stack(bq)                                            # [4, 128]
        vcols = np.concatenate(
            [np.arange(2 * C + h * HS, 2 * C + (h + 1) * HS) for h in heads]
        )
        wv_c = np.ascontiguousarray(w_attn[:, vcols]).astype(mmnp)
        wproj_c = np.ascontiguousarray(w_proj[g * 256 : (g + 1) * 256, :]).astype(mmnp)
        in_maps.append(
            {
                "xt": xts[b],
                "wqk": wqk_c,
                "wv": wv_c,
                "wproj": wproj_c,
                "bqk": np.ascontiguousarray(bqk_c),
                "cosrep": np.ascontiguousarray(cosrep),
                "sinsw": np.ascontiguousarray(sin_sw),
                "ones64": ones64,
            }
        )
    return in_maps


def kernel(x, cos, sin, w_attn, b_attn, w_proj, b_proj, _want_trace=False):
    if "nc" not in _CACHED:
        _CACHED["nc"] = _build_nc()
    nc = _CACHED["nc"]
    in_maps = _prep_core_inputs(x, cos, sin, w_attn, b_attn, w_proj)
    res = run_bass_kernel_spmd(
        nc, in_maps, core_ids=list(range(NCORES)), trace=_want_trace
    )
    _CACHED["last_result"] = res
    b_proj = np.asarray(b_proj, dtype=np.float32)
    # v-bias folds out of attention (softmax rows sum to 1): it contributes a
    # constant b_v @ w_proj to every output row, added here with b_proj.
    bv = np.asarray(b_attn, dtype=np.float32)[2 * C : 3 * C]
    bias_full = b_proj + bv @ np.asarray(w_proj, dtype=np.float32)
    out = np.empty((B, T, C), np.float32)
    for b in range(B):
        acc = res.results[b * 4]["yout"].astype(np.float32).copy()
        for g in range(1, 4):
            acc += res.results[b * 4 + g]["yout"]
        out[b] = acc + bias_full[None, :]
    return out


# revision 13
# speedup vs baseline: 1.3192x; 1.0905x over previous
"""Causal self-attention (B=2, T=2048, C=1024, H=16) on 8 TRN2 NeuronCores.

Sharding: 8 cores = 2 batches x 4 head-groups (4 heads each).
Each core computes qkv projection for its heads, attention, and a partial
output projection (its rows of w_proj); the host sums the 4 partials per
batch and adds b_proj.

Device-side structure (v2, fused pipeline):
  - One software-pipelined loop over four 512-token chunks.  For chunk ci
    the attention i-chunk (scores -> exp -> mask -> AV) is emitted on the
    tensor queue INTERLEAVED with the qkv projection of chunk ci+1 (and,
    for the last chunk, with the output projection of earlier chunks) so
    the PE never idles waiting for the scalar-engine exp and stays at its
    2.4 GHz p-state.
  - Layouts as v1: x fed transposed; q,k produced transposed ([d, t]) so
    scores are computed transposed and softmax needs no reductions (the
    AV matmul with a ones-augmented V yields the denominators); v natural.
  - Diagonal-tile trimming: matmuls/exp on diagonal j-blocks only cover
    the causal column range (c0 = 128k), so fully-masked regions are never
    computed; affine_select only handles the [128,128] partial blocks.
"""

import sys
import os

for _p in ("/opt/trn_rl_repo", "/root/.axon_site/_ro/trn_rl_repo"):
    if os.path.isdir(_p) and _p not in sys.path:
        sys.path.insert(0, _p)

import numpy as np
import concourse.bass as bass
import concourse.mybir as mybir
import concourse.tile as tile
from concourse import bacc
from concourse.bass_utils import run_bass_kernel_spmd

B, T, C, H = 2, 2048, 1024, 16
HS = C // H          # 64
HALF = HS // 2       # 32
NCORES = 8
NH = 4               # heads per core
TCH = 512            # t-chunk for projections / i-chunk for attention
NCH = T // TCH       # 4 chunks
CB = C // 128        # 8 contraction blocks
NTB = T // 128       # 16 t/j blocks
F32 = mybir.dt.float32
F32R = mybir.dt.float32r
BF16 = mybir.dt.bfloat16
USE_BF16 = os.environ.get("KERNEL_BF16", "1") == "1"
MMD = BF16 if USE_BF16 else F32R
AF = mybir.ActivationFunctionType
ALU = mybir.AluOpType

_CACHED = {}


def _build_nc():
    nc = bacc.Bacc("TRN2", target_bir_lowering=False, debug=False)

    xt = nc.dram_tensor("xt", [C, T], MMD, kind="ExternalInput").ap()
    wqk = nc.dram_tensor("wqk", [C, 512], MMD, kind="ExternalInput").ap()
    wv = nc.dram_tensor("wv", [C, 256], MMD, kind="ExternalInput").ap()
    wproj = nc.dram_tensor("wproj", [256, C], MMD, kind="ExternalInput").ap()
    bqk = nc.dram_tensor("bqk", [4, 128], F32, kind="ExternalInput").ap()
    cosrep = nc.dram_tensor("cosrep", [128, T], F32, kind="ExternalInput").ap()
    sinsw = nc.dram_tensor("sinsw", [128, T], F32, kind="ExternalInput").ap()
    ones64 = nc.dram_tensor("ones64", [128, 64], F32, kind="ExternalInput").ap()
    yout = nc.dram_tensor("yout", [T, C], F32, kind="ExternalOutput").ap()

    with tile.TileContext(nc) as tc:
        with (
            tc.tile_pool(name="const", bufs=1) as const,
            tc.tile_pool(name="persist", bufs=1) as persist,
            tc.tile_pool(name="work", bufs=2) as work,
            tc.tile_pool(name="xtp", bufs=2) as xtp,
            tc.tile_pool(name="attnp", bufs=6) as attnp,
            tc.tile_pool(name="ps", bufs=1, space="PSUM") as ps,
        ):
            # ---- constant loads (wqk/xt0 first, per-cb, so the first matmuls
            # can start on partial loads) ----------------------------------------
            wqk_sb = const.tile([128, CB * 512], MMD)
            xt_tiles = {}
            xt0_t = xtp.tile([128, CB * TCH], MMD, tag="xt", bufs=2, name="xt0")
            for cb in range(CB):
                nc.scalar.dma_start(
                    out=wqk_sb[:, cb * 512 : (cb + 1) * 512],
                    in_=wqk[cb * 128 : (cb + 1) * 128, :],
                )
                nc.sync.dma_start(
                    out=xt0_t[:, cb * TCH : (cb + 1) * TCH],
                    in_=xt[cb * 128 : (cb + 1) * 128, 0:TCH],
                )
            xt_tiles[0] = xt0_t
            cos_sb = const.tile([128, T], F32)
            nc.scalar.dma_start(out=cos_sb, in_=cosrep)
            sin_sb = const.tile([128, T], F32)
            nc.scalar.dma_start(out=sin_sb, in_=sinsw)
            ones_sb = const.tile([128, 64], F32)
            nc.scalar.dma_start(out=ones_sb, in_=ones64)
            bqk_sb = const.tile([128, 4], F32)
            for mt in range(4):
                nc.scalar.dma_start(out=bqk_sb[:, mt : mt + 1], in_=bqk[mt, :][:, None])
            wv_sb = const.tile([128, CB * 256], MMD)
            nc.scalar.dma_start(
                out=wv_sb.rearrange("p (cb m) -> p cb m", cb=CB),
                in_=wv.rearrange("(cb p) m -> p cb m", p=128),
            )
            # wproj is only needed from the last attention chunk on
            wproj_sb = const.tile([128, 2 * C], MMD)

            # ---- persistent intermediates --------------------------------------
            qt_sb = persist.tile([128, 2 * T], MMD)   # [Q01 | Q23], [d(2 heads), t]
            kt_sb = persist.tile([128, 2 * T], MMD)
            v_sb = persist.tile([128, NTB * 260], MMD)  # per j-block: 4x(64 v + 1 one)
            ctx0 = persist.tile([128, T], MMD)        # heads 0,1 ctxT
            ctx1 = persist.tile([128, T], MMD)        # heads 2,3 ctxT

            # ones columns of v_sb (once)
            nc.vector.tensor_copy(
                v_sb.rearrange("p (tb h d) -> p tb h d", tb=NTB, h=4)[:, :, :, 64:65],
                ones_sb.rearrange("p (a b c) -> p a b c", a=NTB, b=4),
            )

            # ---- emission helpers ----------------------------------------------
            def emit_xt_dma(ci):
                xt_t = xtp.tile([128, CB * TCH], MMD, tag="xt", bufs=2, name=f"xt{ci}")
                for cb in range(CB):
                    nc.sync.dma_start(
                        out=xt_t[:, cb * TCH : (cb + 1) * TCH],
                        in_=xt[cb * 128 : (cb + 1) * 128, ci * TCH : (ci + 1) * TCH],
                    )
                xt_tiles[ci] = xt_t

            def emit_projqk_mt(ci, mt):
                # M-tiles: 0=Q(h0,h1) 1=Q(h2,h3) 2=K(h0,h1) 3=K(h2,h3)
                xt_t = xt_tiles[ci]
                tsl = slice(ci * TCH, (ci + 1) * TCH)
                pq = ps.tile([128, TCH], F32, tag="sm", bufs=2, name=f"pq{ci}_{mt}")
                for cb in range(CB):
                    nc.tensor.matmul(
                        pq,
                        lhsT=wqk_sb[:, cb * 512 + mt * 128 : cb * 512 + (mt + 1) * 128],
                        rhs=xt_t[:, cb * TCH : (cb + 1) * TCH],
                        start=(cb == 0),
                        stop=(cb == CB - 1),
                    )
                # rope: out = (pq+b)*cos + swap(pq+b)*sin.  Two psum readers
                # only (qb eviction + m1) so the sm slot frees fast.
                m1 = work.tile([128, TCH], F32, tag="m1", bufs=2, name=f"m1_{ci}_{mt}")
                nc.vector.scalar_tensor_tensor(
                    out=m1, in0=pq, scalar=bqk_sb[:, mt : mt + 1],
                    in1=cos_sb[:, tsl], op0=ALU.add, op1=ALU.mult,
                )
                qb = work.tile([128, TCH], F32, tag="qb", bufs=2, name=f"qb{ci}_{mt}")
                nc.scalar.activation(
                    qb, pq, AF.Identity, bias=bqk_sb[:, mt : mt + 1], scale=1.0
                )
                swp = work.tile([128, TCH], F32, tag="swp", bufs=2, name=f"swp{ci}_{mt}")
                for dst0, src0 in ((0, 32), (32, 0), (64, 96), (96, 64)):
                    nc.vector.tensor_mul(
                        swp[dst0 : dst0 + 32, :],
                        qb[src0 : src0 + 32, :],
                        sin_sb[src0 : src0 + 32, tsl],
                    )
                dest = qt_sb if mt < 2 else kt_sb
                dcol = (mt % 2) * T + ci * TCH
                nc.vector.tensor_add(dest[:, dcol : dcol + TCH], m1, swp)

            def emit_projv_half(ci, h):
                # v natural layout [t, d]: two 128-t-blocks per psum tile
                xt_t = xt_tiles[ci]
                pv = ps.tile([128, TCH], F32, tag="sm", bufs=2, name=f"pv{ci}_{h}")
                for sub in range(2):
                    tl = h * 2 + sub            # t-block within chunk
                    tb = ci * 4 + tl            # global 128-t block
                    for cb in range(CB):
                        nc.tensor.matmul(
                            pv[:, sub * 256 : (sub + 1) * 256],
                            lhsT=xt_t[:, cb * TCH + tl * 128 : cb * TCH + (tl + 1) * 128],
                            rhs=wv_sb[:, cb * 256 : (cb + 1) * 256],
                            start=(cb == 0),
                            stop=(cb == CB - 1),
                        )
                for sub in range(2):
                    tb = ci * 4 + h * 2 + sub
                    nc.scalar.copy(
                        v_sb[:, tb * 260 : tb * 260 + 260].rearrange(
                            "p (h d) -> p h d", h=4
                        )[:, :, 0:64],
                        pv[:, sub * 256 : (sub + 1) * 256].rearrange(
                            "p (h d) -> p h d", h=4
                        ),
                    )

            def emit_outproj_tb(tb):
                for ncol in range(2):
                    yp = ps.tile([128, TCH], F32, tag="sm", bufs=2, name=f"yp{tb}_{ncol}")
                    for cb in range(2):
                        ctx_t = ctx0 if cb == 0 else ctx1
                        nc.tensor.matmul(
                            yp,
                            lhsT=ctx_t[:, tb * 128 : (tb + 1) * 128],
                            rhs=wproj_sb[:, cb * C + ncol * 512 : cb * C + (ncol + 1) * 512],
                            start=(cb == 0),
                            stop=(cb == 1),
                        )
                    ysb = work.tile([128, TCH], F32, tag="ysb", bufs=2, name=f"ysb{tb}_{ncol}")
                    nc.vector.tensor_copy(ysb, yp)
                    nc.sync.dma_start(
                        out=yout[tb * 128 : (tb + 1) * 128, ncol * 512 : (ncol + 1) * 512],
                        in_=ysb,
                    )

            # ---- attention chunk with interleaved filler pieces ----------------
            def emit_attention_chunk(ci, fillers):
                npull = [0]
                nslots = 4 * (ci + 1)

                def pull(slot_done):
                    tgt = (slot_done * len(fillers) + nslots - 1) // nslots
                    while npull[0] < min(tgt, len(fillers)):
                        fillers[npull[0]]()
                        npull[0] += 1

                njb = 4 * (ci + 1)
                isl = slice(ci * TCH, (ci + 1) * TCH)
                for pair in range(2):
                    qt_p = qt_sb[:, pair * T : (pair + 1) * T]
                    kt_p = kt_sb[:, pair * T : (pair + 1) * T]
                    ctx_p = ctx0 if pair == 0 else ctx1
                    ctxps = [
                        ps.tile([65, TCH], F32, tag="ctx", bufs=2, name=f"cp{pair}_{ci}_{hh}")
                        for hh in range(2)
                    ]

                    def trim(jb):
                        # causal column trim for diagonal j-blocks; fp32r
                        # clamps to keep matmul free dim >= 256 (full-rate
                        # floor), bf16 has no such cliff
                        lim = 384 if USE_BF16 else 256
                        return min(max(0, 128 * (jb - 4 * ci)), lim)

                    def emit_av(at_pair, duo):
                        for hh in range(2):
                            for half in range(2):
                                jb = duo * 2 + half
                                c0 = trim(jb)
                                h_loc = pair * 2 + hh
                                nc.tensor.matmul(
                                    ctxps[hh][:, c0:TCH],
                                    lhsT=v_sb[:, jb * 260 + h_loc * 65 : jb * 260 + (h_loc + 1) * 65],
                                    rhs=at_pair[hh][:, half * 512 + c0 : (half + 1) * 512],
                                    start=(jb == 0),
                                    stop=(jb == njb - 1),
                                )

                    pending = []
                    for duo in range(njb // 2):
                        st = [
                            ps.tile([128, 1024], F32, tag="st", bufs=2, name=f"st{pair}_{ci}_{duo}_{hh}")
                            for hh in range(2)
                        ]
                        # interleave the two heads' QK matmuls (disjoint row
                        # strips 0-63 / 64-127 -> HW runs them concurrently)
                        for half in range(2):
                            jb = duo * 2 + half
                            c0 = trim(jb)
                            for hh in range(2):
                                nc.tensor.matmul(
                                    st[hh][:, half * 512 + c0 : (half + 1) * 512],
                                    lhsT=kt_p[hh * 64 : (hh + 1) * 64, jb * 128 : (jb + 1) * 128],
                                    rhs=qt_p[hh * 64 : (hh + 1) * 64, ci * TCH + c0 : (ci + 1) * TCH],
                                    start=True,
                                    stop=True,
                                )
                        if len(pending) >= 1:
                            emit_av(*pending.pop(0))
                        at_pair = []
                        for hh in range(2):
                            at = attnp.tile([128, 1024], MMD, tag="attn", bufs=6, name=f"at{pair}_{ci}_{duo}_{hh}")
                            if duo < 2 * ci:  # fully off-diagonal duo
                                nc.scalar.activation(at, st[hh], AF.Exp, scale=0.125)
                            else:
                                # diagonal: exp only the causal column ranges,
                                # then zero j>i up through the partial block
                                for half in range(2):
                                    jb = duo * 2 + half
                                    k = jb - 4 * ci
                                    c0 = trim(jb)
                                    nc.scalar.activation(
                                        at[:, half * 512 + c0 : (half + 1) * 512],
                                        st[hh][:, half * 512 + c0 : (half + 1) * 512],
                                        AF.Exp,
                                        scale=0.125,
                                    )
                                    sel = 128 * (k + 1) - c0
                                    nc.gpsimd.affine_select(
                                        out=at[:, half * 512 + c0 : half * 512 + c0 + sel],
                                        in_=at[:, half * 512 + c0 : half * 512 + c0 + sel],
                                        compare_op=ALU.is_ge,
                                        fill=0.0,
                                        base=c0 - 128 * k,
                                        channel_multiplier=-1,
                                        pattern=[[1, sel]],
                                    )
                            at_pair.append(at)
                        pending.append((at_pair, duo))
                        pull(pair * (njb // 2) + duo + 1)
                    for p in pending:
                        emit_av(*p)

                    # normalize: ctx[d, i] /= denom[i] (denom = row 64)
                    for hh in range(2):
                        # single-op psum eviction: ctx slot frees immediately;
                        # normalization continues SBUF-side off the PE path
                        ctxu = work.tile([64, TCH], F32, tag="ctxu", bufs=2, name=f"cu{pair}_{ci}_{hh}")
                        nc.vector.tensor_copy(ctxu, ctxps[hh][0:64, :])
                        dn = work.tile([1, TCH], F32, tag="dnrow", bufs=2, name=f"dn{pair}_{ci}_{hh}")
                        nc.vector.tensor_copy(dn, ctxps[hh][64:65, :])
                        rc = work.tile([1, TCH], F32, tag="recip", bufs=1, name=f"rc{pair}_{ci}_{hh}")
                        nc.vector.reciprocal_approx_fast(out=rc, in_=dn)
                        bcast = work.tile([64, TCH], F32, tag="bcast", bufs=2, name=f"bcast{pair}_{ci}_{hh}")
                        nc.gpsimd.partition_broadcast(bcast, rc)
                        nc.vector.tensor_mul(
                            ctx_p[hh * 64 : (hh + 1) * 64, isl],
                            ctxu,
                            bcast,
                        )
                pull(nslots)  # drain any leftover fillers

            # ---- fused pipeline ------------------------------------------------
            def proj_pieces(ci):
                return (
                    [lambda mt=mt: emit_projqk_mt(ci, mt) for mt in range(4)]
                    + [lambda h=h: emit_projv_half(ci, h) for h in range(2)]
                )

            # lead-in: chunk 0 projection runs alone; chunk 1 xt prefetch
            emit_xt_dma(1)
            for f in proj_pieces(0):
                f()
            for ci in range(NCH):
                if ci + 2 < NCH:
                    emit_xt_dma(ci + 2)
                if ci == 1:
                    nc.scalar.dma_start(
                        out=wproj_sb.rearrange("p (cb n) -> p cb n", cb=2),
                        in_=wproj.rearrange("(cb p) n -> p cb n", p=128),
                    )
                if ci + 1 < NCH:
                    fillers = proj_pieces(ci + 1)
                else:
                    # last chunk: fill PE gaps with earlier chunks' out-proj
                    fillers = [
                        (lambda tb=tb: emit_outproj_tb(tb))
                        for tb in range((NCH - 1) * 4)
                    ]
                emit_attention_chunk(ci, fillers)
            for tb in range((NCH - 1) * 4, NCH * 4):
                emit_outproj_tb(tb)

    nc.compile()
    return nc


def _prep_core_inputs(x, cos, sin, w_attn, b_attn, w_proj):
    """Build the 8 per-core input maps (host-side shard/reorder)."""
    import ml_dtypes
    mmnp = ml_dtypes.bfloat16 if USE_BF16 else np.float32
    x = np.asarray(x, dtype=np.float32)
    cos = np.asarray(cos, dtype=np.float32).reshape(T, HALF)
    sin = np.asarray(sin, dtype=np.float32).reshape(T, HALF)
    w_attn = np.asarray(w_attn, dtype=np.float32)
    b_attn = np.asarray(b_attn, dtype=np.float32)
    w_proj = np.asarray(w_proj, dtype=np.float32)

    cosT = np.ascontiguousarray(cos.T)               # [32, T]
    sinT = np.ascontiguousarray(sin.T)
    cosrep = np.tile(cosT, (4, 1))                   # [128, T]
    sin_sw = np.concatenate([sinT, -sinT, sinT, -sinT], axis=0)
    ones64 = np.ones((128, 64), np.float32)

    xts = [np.ascontiguousarray(x[b].T).astype(mmnp) for b in range(B)]  # [C, T] each

    in_maps = []
    for core in range(NCORES):
        b = core // 4
        g = core % 4
        heads = [4 * g + i for i in range(NH)]
        # q/k column blocks: M-tiles [Q(h0,h1), Q(h2,h3), K(h0,h1), K(h2,h3)]
        qcols, bq = [], []
        for mt, (base, hs) in enumerate(
            [(0, heads[0:2]), (0, heads[2:4]), (C, heads[0:2]), (C, heads[2:4])]
        ):
            cols = np.concatenate([np.arange(base + h * HS, base + (h + 1) * HS) for h in hs])
            qcols.append(cols)
            bq.append(b_attn[cols])
        wqk_c = np.ascontiguousarray(w_attn[:, np.concatenate(qcols)]).astype(mmnp)
        bqk_c = np.stack(bq)                                            # [4, 128]
        vcols = np.concatenate(
            [np.arange(2 * C + h * HS, 2 * C + (h + 1) * HS) for h in heads]
        )
        wv_c = np.ascontiguousarray(w_attn[:, vcols]).astype(mmnp)
        wproj_c = np.ascontiguousarray(w_proj[g * 256 : (g + 1) * 256, :]).astype(mmnp)
        in_maps.append(
            {
                "xt": xts[b],
                "wqk": wqk_c,
                "wv": wv_c,
                "wproj": wproj_c,
                "bqk": np.ascontiguousarray(bqk_c),
                "cosrep": np.ascontiguousarray(cosrep),
                "sinsw": np.ascontiguousarray(sin_sw),
                "ones64": ones64,
            }
        )
    return in_maps


def kernel(x, cos, sin, w_attn, b_attn, w_proj, b_proj, _want_trace=False):
    if "nc" not in _CACHED:
        _CACHED["nc"] = _build_nc()
    nc = _CACHED["nc"]
    in_maps = _prep_core_inputs(x, cos, sin, w_attn, b_attn, w_proj)
    res = run_bass_kernel_spmd(
        nc, in_maps, core_ids=list(range(NCORES)), trace=_want_trace
    )
    _CACHED["last_result"] = res
    b_proj = np.asarray(b_proj, dtype=np.float32)
    # v-bias folds out of attention (softmax rows sum to 1): it contributes a
    # constant b_v @ w_proj to every output row, added here with b_proj.
    bv = np.asarray(b_attn, dtype=np.float32)[2 * C : 3 * C]
    bias_full = b_proj + bv @ np.asarray(w_proj, dtype=np.float32)
    out = np.empty((B, T, C), np.float32)
    for b in range(B):
        acc = res.results[b * 4]["yout"].astype(np.float32).copy()
        for g in range(1, 4):
            acc += res.results[b * 4 + g]["yout"]
        out[b] = acc + bias_full[None, :]
    return out


# revision 21
# speedup vs baseline: 1.3571x; 1.0287x over previous
"""Causal self-attention (B=2, T=2048, C=1024, H=16) on 8 TRN2 NeuronCores.

Sharding: 8 cores = 2 batches x 4 head-groups (4 heads each).
Each core computes qkv projection for its heads, attention, and a partial
output projection (its rows of w_proj); the host sums the 4 partials per
batch and adds b_proj.

Device-side structure (v2, fused pipeline):
  - One software-pipelined loop over four 512-token chunks.  For chunk ci
    the attention i-chunk (scores -> exp -> mask -> AV) is emitted on the
    tensor queue INTERLEAVED with the qkv projection of chunk ci+1 (and,
    for the last chunk, with the output projection of earlier chunks) so
    the PE never idles waiting for the scalar-engine exp and stays at its
    2.4 GHz p-state.
  - Layouts as v1: x fed transposed; q,k produced transposed ([d, t]) so
    scores are computed transposed and softmax needs no reductions (the
    AV matmul with a ones-augmented V yields the denominators); v natural.
  - Diagonal-tile trimming: matmuls/exp on diagonal j-blocks only cover
    the causal column range (c0 = 128k), so fully-masked regions are never
    computed; affine_select only handles the [128,128] partial blocks.
"""

import sys
import os

for _p in ("/opt/trn_rl_repo", "/root/.axon_site/_ro/trn_rl_repo"):
    if os.path.isdir(_p) and _p not in sys.path:
        sys.path.insert(0, _p)

import numpy as np
import concourse.bass as bass
import concourse.mybir as mybir
import concourse.tile as tile
from concourse import bacc
from concourse.bass_utils import run_bass_kernel_spmd

B, T, C, H = 2, 2048, 1024, 16
HS = C // H          # 64
HALF = HS // 2       # 32
NCORES = 8
NH = 4               # heads per core
TCH = 512            # t-chunk for projections / i-chunk for attention
NCH = T // TCH       # 4 chunks
CB = C // 128        # 8 contraction blocks
NTB = T // 128       # 16 t/j blocks
F32 = mybir.dt.float32
F32R = mybir.dt.float32r
BF16 = mybir.dt.bfloat16
USE_BF16 = os.environ.get("KERNEL_BF16", "1") == "1"
MMD = BF16 if USE_BF16 else F32R
AF = mybir.ActivationFunctionType
ALU = mybir.AluOpType

_CACHED = {}


def _build_nc():
    nc = bacc.Bacc("TRN2", target_bir_lowering=False, debug=False)

    xt = nc.dram_tensor("xt", [C, T], MMD, kind="ExternalInput").ap()
    wqk = nc.dram_tensor("wqk", [C, 512], MMD, kind="ExternalInput").ap()
    wv = nc.dram_tensor("wv", [C, 256], MMD, kind="ExternalInput").ap()
    wproj = nc.dram_tensor("wproj", [256, C], MMD, kind="ExternalInput").ap()
    bqk = nc.dram_tensor("bqk", [4, 128], F32, kind="ExternalInput").ap()
    # cosrep: cos duplicated on both rope halves; sinswp: sign-patterned sin
    # PRE-PERMUTED by the rope partner map (see _prep_core_inputs)
    cosrep = nc.dram_tensor("cosrep", [128, T], MMD, kind="ExternalInput").ap()
    sinswp = nc.dram_tensor("sinswp", [128, T], MMD, kind="ExternalInput").ap()
    permm = nc.dram_tensor("permm", [128, 128], MMD, kind="ExternalInput").ap()
    ones64 = nc.dram_tensor("ones64", [128, 64], F32, kind="ExternalInput").ap()
    yout = nc.dram_tensor("yout", [T, C], F32, kind="ExternalOutput").ap()

    with tile.TileContext(nc) as tc:
        with (
            tc.tile_pool(name="const", bufs=1) as const,
            tc.tile_pool(name="persist", bufs=1) as persist,
            tc.tile_pool(name="work", bufs=2) as work,
            tc.tile_pool(name="xtp", bufs=2) as xtp,
            tc.tile_pool(name="attnp", bufs=6) as attnp,
            tc.tile_pool(name="ps", bufs=1, space="PSUM") as ps,
        ):
            # ---- constant loads (wqk/xt0 first, per-cb, so the first matmuls
            # can start on partial loads) ----------------------------------------
            wqk_sb = const.tile([128, CB * 512], MMD)
            xt_tiles = {}
            xt0_t = xtp.tile([128, CB * TCH], MMD, tag="xt", bufs=2, name="xt0")
            cos_sb = const.tile([128, T], MMD)
            sin_sb = const.tile([128, T], MMD)
            perm_sb = const.tile([128, 128], MMD)
            ones_sb = const.tile([128, 64], F32)
            bqk_sb = const.tile([128, 4], F32)
            wv_sb = const.tile([128, CB * 256], MMD)
            for cb in range(CB):
                nc.scalar.dma_start(
                    out=wqk_sb[:, cb * 512 : (cb + 1) * 512],
                    in_=wqk[cb * 128 : (cb + 1) * 128, :],
                )
                nc.sync.dma_start(
                    out=xt0_t[:, cb * TCH : (cb + 1) * TCH],
                    in_=xt[cb * 128 : (cb + 1) * 128, 0:TCH],
                )
                # interleave the small-but-urgent consts behind the first cbs
                if cb == 0:
                    nc.scalar.dma_start(out=cos_sb, in_=cosrep)
                    nc.scalar.dma_start(out=sin_sb, in_=sinswp)
                    nc.scalar.dma_start(out=perm_sb, in_=permm)
                    nc.scalar.dma_start(out=ones_sb, in_=ones64)
                    for mt in range(4):
                        nc.scalar.dma_start(
                            out=bqk_sb[:, mt : mt + 1], in_=bqk[mt, :][:, None]
                        )
                if cb == 1:
                    nc.scalar.dma_start(
                        out=wv_sb.rearrange("p (cb m) -> p cb m", cb=CB),
                        in_=wv.rearrange("(cb p) m -> p cb m", p=128),
                    )
            xt_tiles[0] = xt0_t
            # wproj is only needed from the last attention chunk on
            wproj_sb = const.tile([128, 2 * C], MMD)

            # ---- persistent intermediates --------------------------------------
            qt_sb = persist.tile([128, 2 * T], MMD)   # [Q01 | Q23], [d(2 heads), t]
            kt_sb = persist.tile([128, 2 * T], MMD)
            v_sb = persist.tile([128, NTB * 260], MMD)  # per j-block: 4x(64 v + 1 one)
            ctx0 = persist.tile([128, T], MMD)        # heads 0,1 ctxT
            ctx1 = persist.tile([128, T], MMD)        # heads 2,3 ctxT

            # ones columns of v_sb (once)
            nc.vector.tensor_copy(
                v_sb.rearrange("p (tb h d) -> p tb h d", tb=NTB, h=4)[:, :, :, 64:65],
                ones_sb.rearrange("p (a b c) -> p a b c", a=NTB, b=4),
            )

            # ---- emission helpers ----------------------------------------------
            def emit_xt_dma(ci):
                xt_t = xtp.tile([128, CB * TCH], MMD, tag="xt", bufs=2, name=f"xt{ci}")
                for cb in range(CB):
                    nc.sync.dma_start(
                        out=xt_t[:, cb * TCH : (cb + 1) * TCH],
                        in_=xt[cb * 128 : (cb + 1) * 128, ci * TCH : (ci + 1) * TCH],
                    )
                xt_tiles[ci] = xt_t

            def emit_projqk_mms(ci, mt):
                # M-tiles: 0=Q(h0,h1) 1=Q(h2,h3) 2=K(h0,h1) 3=K(h2,h3)
                xt_t = xt_tiles[ci]
                pq = ps.tile([128, TCH], F32, tag="sm", bufs=2, name=f"pq{ci}_{mt}")
                for cb in range(CB):
                    nc.tensor.matmul(
                        pq,
                        lhsT=wqk_sb[:, cb * 512 + mt * 128 : cb * 512 + (mt + 1) * 128],
                        rhs=xt_t[:, cb * TCH : (cb + 1) * TCH],
                        start=(cb == 0),
                        stop=(cb == CB - 1),
                    )
                return pq

            def emit_rope_pre(ci, mt, pq):
                # rope: out = (pq+b)*cos + P @ ((pq+b)*sinP) where P is the
                # half-swap permutation (sinP carries the sign pattern and is
                # pre-permuted so the PE matmul does the partition swap).
                # Two psum readers only (qb eviction + m1) -> sm slot frees fast.
                tsl = slice(ci * TCH, (ci + 1) * TCH)
                m1 = work.tile([128, TCH], F32, tag="m1", bufs=2, name=f"m1_{ci}_{mt}")
                nc.vector.scalar_tensor_tensor(
                    out=m1, in0=pq, scalar=bqk_sb[:, mt : mt + 1],
                    in1=cos_sb[:, tsl], op0=ALU.add, op1=ALU.mult,
                )
                qb = work.tile([128, TCH], F32, tag="qb", bufs=2, name=f"qb{ci}_{mt}")
                nc.scalar.activation(
                    qb, pq, AF.Identity, bias=bqk_sb[:, mt : mt + 1], scale=1.0
                )
                u = work.tile([128, TCH], MMD, tag="u", bufs=2, name=f"u{ci}_{mt}")
                nc.vector.tensor_mul(u, qb, sin_sb[:, tsl])
                return m1, u

            def emit_rope_fin(ci, mt, m1, u):
                # deferred so the swap matmul's input (u) is ready when the PE
                # reaches it (pipelined one mt behind the projection matmuls)
                up = ps.tile([128, TCH], F32, tag="sm", bufs=2, name=f"up{ci}_{mt}")
                nc.tensor.matmul(up, lhsT=perm_sb, rhs=u, start=True, stop=True)
                dest = qt_sb if mt < 2 else kt_sb
                dcol = (mt % 2) * T + ci * TCH
                nc.vector.tensor_add(dest[:, dcol : dcol + TCH], m1, up)

            def emit_projv_half(ci, h):
                # v natural layout [t, d]: two 128-t-blocks per psum tile
                xt_t = xt_tiles[ci]
                pv = ps.tile([128, TCH], F32, tag="sm", bufs=2, name=f"pv{ci}_{h}")
                for sub in range(2):
                    tl = h * 2 + sub            # t-block within chunk
                    tb = ci * 4 + tl            # global 128-t block
                    for cb in range(CB):
                        nc.tensor.matmul(
                            pv[:, sub * 256 : (sub + 1) * 256],
                            lhsT=xt_t[:, cb * TCH + tl * 128 : cb * TCH + (tl + 1) * 128],
                            rhs=wv_sb[:, cb * 256 : (cb + 1) * 256],
                            start=(cb == 0),
                            stop=(cb == CB - 1),
                        )
                tb = ci * 4 + h * 2
                nc.scalar.copy(
                    v_sb[:, tb * 260 : (tb + 2) * 260].rearrange(
                        "p (b h d) -> p b h d", b=2, h=4
                    )[:, :, :, 0:64],
                    pv.rearrange("p (b h d) -> p b h d", b=2, h=4),
                )

            def emit_outproj_tb(tb):
                for ncol in range(2):
                    yp = ps.tile([128, TCH], F32, tag="sm", bufs=2, name=f"yp{tb}_{ncol}")
                    for cb in range(2):
                        ctx_t = ctx0 if cb == 0 else ctx1
                        nc.tensor.matmul(
                            yp,
                            lhsT=ctx_t[:, tb * 128 : (tb + 1) * 128],
                            rhs=wproj_sb[:, cb * C + ncol * 512 : cb * C + (ncol + 1) * 512],
                            start=(cb == 0),
                            stop=(cb == 1),
                        )
                    ysb = work.tile([128, TCH], F32, tag="ysb", bufs=2, name=f"ysb{tb}_{ncol}")
                    nc.vector.tensor_copy(ysb, yp)
                    nc.sync.dma_start(
                        out=yout[tb * 128 : (tb + 1) * 128, ncol * 512 : (ncol + 1) * 512],
                        in_=ysb,
                    )

            # ---- attention chunk with interleaved filler pieces ----------------
            def emit_attention_chunk(ci, fillers):
                npull = [0]
                nslots = 4 * (ci + 1)

                def pull(slot_done):
                    tgt = (slot_done * len(fillers) + nslots - 1) // nslots
                    while npull[0] < min(tgt, len(fillers)):
                        fillers[npull[0]]()
                        npull[0] += 1

                njb = 4 * (ci + 1)
                isl = slice(ci * TCH, (ci + 1) * TCH)
                for pair in range(2):
                    qt_p = qt_sb[:, pair * T : (pair + 1) * T]
                    kt_p = kt_sb[:, pair * T : (pair + 1) * T]
                    ctx_p = ctx0 if pair == 0 else ctx1
                    ctxps = [
                        ps.tile([65, TCH], F32, tag="ctx", bufs=2, name=f"cp{pair}_{ci}_{hh}")
                        for hh in range(2)
                    ]

                    def trim(jb):
                        # causal column trim for diagonal j-blocks; fp32r
                        # clamps to keep matmul free dim >= 256 (full-rate
                        # floor), bf16 has no such cliff
                        lim = 384 if USE_BF16 else 256
                        return min(max(0, 128 * (jb - 4 * ci)), lim)

                    def emit_av(at_pair, duo):
                        for hh in range(2):
                            for half in range(2):
                                jb = duo * 2 + half
                                c0 = trim(jb)
                                h_loc = pair * 2 + hh
                                nc.tensor.matmul(
                                    ctxps[hh][:, c0:TCH],
                                    lhsT=v_sb[:, jb * 260 + h_loc * 65 : jb * 260 + (h_loc + 1) * 65],
                                    rhs=at_pair[hh][:, half * 512 + c0 : (half + 1) * 512],
                                    start=(jb == 0),
                                    stop=(jb == njb - 1),
                                )

                    pending = []
                    for duo in range(njb // 2):
                        st = [
                            ps.tile([128, 1024], F32, tag="st", bufs=2, name=f"st{pair}_{ci}_{duo}_{hh}")
                            for hh in range(2)
                        ]
                        # interleave the two heads' QK matmuls (disjoint row
                        # strips 0-63 / 64-127 -> HW runs them concurrently)
                        for half in range(2):
                            jb = duo * 2 + half
                            c0 = trim(jb)
                            for hh in range(2):
                                nc.tensor.matmul(
                                    st[hh][:, half * 512 + c0 : (half + 1) * 512],
                                    lhsT=kt_p[hh * 64 : (hh + 1) * 64, jb * 128 : (jb + 1) * 128],
                                    rhs=qt_p[hh * 64 : (hh + 1) * 64, ci * TCH + c0 : (ci + 1) * TCH],
                                    start=True,
                                    stop=True,
                                )
                        if len(pending) >= 1:
                            emit_av(*pending.pop(0))
                        at_pair = []
                        for hh in range(2):
                            at = attnp.tile([128, 1024], MMD, tag="attn", bufs=6, name=f"at{pair}_{ci}_{duo}_{hh}")
                            if duo < 2 * ci:  # fully off-diagonal duo
                                nc.scalar.activation(at, st[hh], AF.Exp, scale=0.125)
                            else:
                                # diagonal: exp only the causal column ranges,
                                # then zero j>i up through the partial block
                                for half in range(2):
                                    jb = duo * 2 + half
                                    k = jb - 4 * ci
                                    c0 = trim(jb)
                                    nc.scalar.activation(
                                        at[:, half * 512 + c0 : (half + 1) * 512],
                                        st[hh][:, half * 512 + c0 : (half + 1) * 512],
                                        AF.Exp,
                                        scale=0.125,
                                    )
                                    sel = 128 * (k + 1) - c0
                                    nc.gpsimd.affine_select(
                                        out=at[:, half * 512 + c0 : half * 512 + c0 + sel],
                                        in_=at[:, half * 512 + c0 : half * 512 + c0 + sel],
                                        compare_op=ALU.is_ge,
                                        fill=0.0,
                                        base=c0 - 128 * k,
                                        channel_multiplier=-1,
                                        pattern=[[1, sel]],
                                    )
                            at_pair.append(at)
                        pending.append((at_pair, duo))
                        pull(pair * (njb // 2) + duo + 1)
                    for p in pending:
                        emit_av(*p)

                    # normalize: ctx[d, i] /= denom[i] (denom = row 64)
                    for hh in range(2):
                        # single-op psum eviction: ctx slot frees immediately;
                        # normalization continues SBUF-side off the PE path
                        ctxu = work.tile([64, TCH], F32, tag="ctxu", bufs=2, name=f"cu{pair}_{ci}_{hh}")
                        nc.vector.tensor_copy(ctxu, ctxps[hh][0:64, :])
                        dn = work.tile([1, TCH], F32, tag="dnrow", bufs=2, name=f"dn{pair}_{ci}_{hh}")
                        nc.vector.tensor_copy(dn, ctxps[hh][64:65, :])
                        rc = work.tile([1, TCH], F32, tag="recip", bufs=1, name=f"rc{pair}_{ci}_{hh}")
                        nc.vector.reciprocal_approx_fast(out=rc, in_=dn)
                        bcast = work.tile([64, TCH], F32, tag="bcast", bufs=2, name=f"bcast{pair}_{ci}_{hh}")
                        nc.gpsimd.partition_broadcast(bcast, rc)
                        nc.vector.tensor_mul(
                            ctx_p[hh * 64 : (hh + 1) * 64, isl],
                            ctxu,
                            bcast,
                        )
                pull(nslots)  # drain any leftover fillers

            # ---- fused pipeline ------------------------------------------------
            def proj_pieces(ci):
                state = {}

                def mk_qk(mt):
                    def f():
                        pq = emit_projqk_mms(ci, mt)
                        state[mt] = emit_rope_pre(ci, mt, pq)
                        if mt >= 1:
                            emit_rope_fin(ci, mt - 1, *state.pop(mt - 1))
                    return f

                def mk_v(h):
                    def f():
                        emit_projv_half(ci, h)
                        if h == 0:
                            emit_rope_fin(ci, 3, *state.pop(3))
                    return f

                return [mk_qk(mt) for mt in range(4)] + [mk_v(0), mk_v(1)]

            # lead-in: chunk 0 projection runs alone; chunk 1 xt prefetch
            emit_xt_dma(1)
            for f in proj_pieces(0):
                f()
            for ci in range(NCH):
                if ci + 2 < NCH:
                    emit_xt_dma(ci + 2)
                if ci == 1:
                    nc.scalar.dma_start(
                        out=wproj_sb.rearrange("p (cb n) -> p cb n", cb=2),
                        in_=wproj.rearrange("(cb p) n -> p cb n", p=128),
                    )
                if ci + 1 < NCH:
                    fillers = proj_pieces(ci + 1)
                else:
                    # last chunk: fill PE gaps with earlier chunks' out-proj
                    fillers = [
                        (lambda tb=tb: emit_outproj_tb(tb))
                        for tb in range((NCH - 1) * 4)
                    ]
                emit_attention_chunk(ci, fillers)
            for tb in range((NCH - 1) * 4, NCH * 4):
                emit_outproj_tb(tb)

    nc.compile()
    return nc


def _prep_core_inputs(x, cos, sin, w_attn, b_attn, w_proj):
    """Build the 8 per-core input maps (host-side shard/reorder)."""
    import ml_dtypes
    mmnp = ml_dtypes.bfloat16 if USE_BF16 else np.float32
    x = np.asarray(x, dtype=np.float32)
    cos = np.asarray(cos, dtype=np.float32).reshape(T, HALF)
    sin = np.asarray(sin, dtype=np.float32).reshape(T, HALF)
    w_attn = np.asarray(w_attn, dtype=np.float32)
    b_attn = np.asarray(b_attn, dtype=np.float32)
    w_proj = np.asarray(w_proj, dtype=np.float32)

    cosT = np.ascontiguousarray(cos.T)               # [32, T]
    sinT = np.ascontiguousarray(sin.T)
    cosrep = np.tile(cosT, (4, 1)).astype(mmnp)      # [128, T]
    sin_sw = np.concatenate([sinT, -sinT, sinT, -sinT], axis=0)
    # rope partner map: swap 32-row halves within each 64-row (head) block.
    # u = qb*sin_sw computed row-wise, then the PE matmul with permm moves
    # row partner(p) -> p (swp[p] = u[partner(p)]).
    perm_idx = np.concatenate(
        [np.arange(32, 64), np.arange(0, 32), np.arange(96, 128), np.arange(64, 96)]
    )
    sin_swp = np.ascontiguousarray(sin_sw).astype(mmnp)
    permm = np.zeros((128, 128), np.float32)
    permm[perm_idx, np.arange(128)] = 1.0            # lhsT: out[p] = in[perm_idx[p]]
    permm = permm.astype(mmnp)
    ones64 = np.ones((128, 64), np.float32)

    xts = [np.ascontiguousarray(x[b].T).astype(mmnp) for b in range(B)]  # [C, T] each

    in_maps = []
    for core in range(NCORES):
        b = core // 4
        g = core % 4
        heads = [4 * g + i for i in range(NH)]
        # q/k column blocks: M-tiles [Q(h0,h1), Q(h2,h3), K(h0,h1), K(h2,h3)]
        qcols, bq = [], []
        for mt, (base, hs) in enumerate(
            [(0, heads[0:2]), (0, heads[2:4]), (C, heads[0:2]), (C, heads[2:4])]
        ):
            cols = np.concatenate([np.arange(base + h * HS, base + (h + 1) * HS) for h in hs])
            qcols.append(cols)
            bq.append(b_attn[cols])
        wqk_c = np.ascontiguousarray(w_attn[:, np.concatenate(qcols)]).astype(mmnp)
        bqk_c = np.stack(bq)                                            # [4, 128]
        vcols = np.concatenate(
            [np.arange(2 * C + h * HS, 2 * C + (h + 1) * HS) for h in heads]
        )
        wv_c = np.ascontiguousarray(w_attn[:, vcols]).astype(mmnp)
        wproj_c = np.ascontiguousarray(w_proj[g * 256 : (g + 1) * 256, :]).astype(mmnp)
        in_maps.append(
            {
                "xt": xts[b],
                "wqk": wqk_c,
                "wv": wv_c,
                "wproj": wproj_c,
                "bqk": np.ascontiguousarray(bqk_c),
                "cosrep": np.ascontiguousarray(cosrep),
                "sinswp": sin_swp,
                "permm": permm,
                "ones64": ones64,
            }
        )
    return in_maps


def kernel(x, cos, sin, w_attn, b_attn, w_proj, b_proj, _want_trace=False):
    if "nc" not in _CACHED:
        _CACHED["nc"] = _build_nc()
    nc = _CACHED["nc"]
    in_maps = _prep_core_inputs(x, cos, sin, w_attn, b_attn, w_proj)
    res = run_bass_kernel_spmd(
        nc, in_maps, core_ids=list(range(NCORES)), trace=_want_trace
    )
    _CACHED["last_result"] = res
    b_proj = np.asarray(b_proj, dtype=np.float32)
    # v-bias folds out of attention (softmax rows sum to 1): it contributes a
    # constant b_v @ w_proj to every output row, added here with b_proj.
    bv = np.asarray(b_attn, dtype=np.float32)[2 * C : 3 * C]
    bias_full = b_proj + bv @ np.asarray(w_proj, dtype=np.float32)
    out = np.empty((B, T, C), np.float32)
    for b in range(B):
        acc = res.results[b * 4]["yout"].astype(np.float32).copy()
        for g in range(1, 4):
            acc += res.results[b * 4 + g]["yout"]
        out[b] = acc + bias_full[None, :]
    return out
